# revision 1
# baseline (speedup 1.0000x reference)
import sys

if '/opt/trn_rl_repo' not in sys.path:
    sys.path.insert(0, '/opt/trn_rl_repo')

import numpy as np

G, N1, N2, K = 32, 2048, 2048, 64
N_CORES = 8
GPC = G // N_CORES  # graphs per core

_cache = {}


def _build(m1, m2):
    """Build + bacc-compile the per-core Bass program for maxcounts (m1, m2).

    Per core (GPC graphs, 3 feature tensors):
      inputs  p1  [3, GPC, m1, K]   group-1 rows padded per graph
              p2f [3, GPC, 1, m2*K] group-2 rows padded, flattened per graph
      output  out [3, GPC, 2, m1, m2*K]
    Channel 0 rows are p1[g] broadcast along m2 (DVE copy with stride-0 dim);
    channel 1 rows are p2f[g] broadcast across partitions (gpsimd ucode).
    Both expand into [m1, m2*K] SBUF tiles that stream out as one
    contiguous ~1.7 MB DMA each, keeping HBM writes at line rate.
    """
    from concourse import bacc
    import concourse.tile as tile
    import concourse.mybir as mybir

    m2k = m2 * K
    nc = bacc.Bacc("TRN2", target_bir_lowering=False, debug=False)
    p1 = nc.dram_tensor("p1", [3, GPC, m1, K], mybir.dt.float32, kind="ExternalInput")
    p2f = nc.dram_tensor("p2f", [3, GPC, 1, m2k], mybir.dt.float32, kind="ExternalInput")
    out = nc.dram_tensor("out", [3, GPC, 2, m1, m2k], mybir.dt.float32, kind="ExternalOutput")

    with tile.TileContext(nc) as tc:
        with tc.tile_pool(name="in1", bufs=2) as pin1, \
             tc.tile_pool(name="in2", bufs=2) as pin2, \
             tc.tile_pool(name="exp0", bufs=3) as pexp0, \
             tc.tile_pool(name="exp1", bufs=3) as pexp1:
            for t in range(3):
                for g in range(GPC):
                    t1 = pin1.tile([m1, K], mybir.dt.float32)
                    nc.sync.dma_start(t1[:], p1[t, g])
                    e0 = pexp0.tile([m1, m2k], mybir.dt.float32)
                    nc.vector.tensor_copy(
                        e0[:].rearrange("p (j k) -> p j k", k=K),
                        t1[:].unsqueeze(1).broadcast_to([m1, m2, K]),
                    )
                    nc.sync.dma_start(out[t, g, 0], e0[:])

                    t2 = pin2.tile([1, m2k], mybir.dt.float32)
                    nc.sync.dma_start(t2[:], p2f[t, g])
                    e1 = pexp1.tile([m1, m2k], mybir.dt.float32)
                    nc.gpsimd.partition_broadcast(e1[:], t2[:1])
                    nc.sync.dma_start(out[t, g, 1], e1[:])
    nc.compile()
    return nc


def _pad_groups_np(x, ids, m):
    """[N, K] rows -> [G, m, K] zero-padded per graph (rank within graph)."""
    counts = np.bincount(ids, minlength=G)
    starts = np.cumsum(counts) - counts
    pos = np.arange(ids.shape[0]) - starts[ids]
    outp = np.zeros((G, m, x.shape[1]), x.dtype)
    outp[ids, pos] = x
    return outp


def kernel(**inputs):
    from concourse.bass_utils import run_bass_kernel_spmd

    ids1 = np.asarray(inputs['ids1']).astype(np.int64)
    ids2 = np.asarray(inputs['ids2']).astype(np.int64)
    m1 = int(inputs['maxcount1'])
    m2 = int(inputs['maxcount2'])
    m2k = m2 * K

    xs1 = [np.asarray(inputs[n], dtype=np.float32) for n in ('x_f_1', 'x_e_1', 'x_v_1')]
    xs2 = [np.asarray(inputs[n], dtype=np.float32) for n in ('x_f_2', 'x_e_2', 'x_v_2')]

    pad1 = np.stack([_pad_groups_np(x, ids1, m1) for x in xs1])  # [3, G, m1, K]
    pad2 = np.stack([_pad_groups_np(x, ids2, m2) for x in xs2])  # [3, G, m2, K]
    pad2f = pad2.reshape(3, G, 1, m2k)

    key = (m1, m2)
    if key not in _cache:
        _cache[key] = _build(m1, m2)
    nc = _cache[key]

    in_maps = [
        {
            "p1": np.ascontiguousarray(pad1[:, c * GPC:(c + 1) * GPC]),
            "p2f": np.ascontiguousarray(pad2f[:, c * GPC:(c + 1) * GPC]),
        }
        for c in range(N_CORES)
    ]
    res = run_bass_kernel_spmd(nc, in_maps, core_ids=list(range(N_CORES)))

    full = np.empty((3, G, 2, m1, m2, K), np.float32)
    for c in range(N_CORES):
        full[:, c * GPC:(c + 1) * GPC] = res.results[c]["out"].reshape(
            3, GPC, 2, m1, m2, K)
    return full[0], full[1], full[2]


# revision 2
# speedup vs baseline: 1.1284x; 1.1284x over previous
import sys

if '/opt/trn_rl_repo' not in sys.path:
    sys.path.insert(0, '/opt/trn_rl_repo')

import numpy as np

G, N1, N2, K = 32, 2048, 2048, 64
N_CORES = 8
GPC = G // N_CORES  # graphs per core

_cache = {}


def _build(m1, m2):
    """Per-core Bass program (act_ring plan).

    inputs  p1  [3, GPC, m1, K]    group-1 padded rows
            p2f [3, GPC, 1, m2*K]  group-2 padded rows, flattened
    output  out [3, GPC, 2, m1, m2*K]

    ch0 rows (p1 broadcast along j) expand on DVE; ch1 rows (p2f repeated
    per i) expand on GPSIMD partition-broadcast, except 2 tiles that go to
    DVE via paired-row broadcast ([m2//2, 2K] pairs -> 512B-aligned strided
    writes) to keep GPSIMD under the DMA shadow. ACT issues half the DMAs
    (second HWDGE ring) and does no compute, so its ring never stalls
    behind copies. The odd last j-row of paired tiles is a stride-0-source
    HBM->HBM DMA with no engine producer.
    """
    from concourse import bacc
    import concourse.tile as tile
    import concourse.mybir as mybir

    F32 = mybir.dt.float32
    m2k = m2 * K
    npair = m2 // 2          # full 2K-elem pairs, e.g. 41
    paired = npair * 2 * K   # elems covered by pairs, e.g. 5248
    half1 = m1 // 2          # i-split for paired tiles, e.g. 41
    N_PAIRED = 2

    nc = bacc.Bacc("TRN2", target_bir_lowering=False, debug=False)
    p1 = nc.dram_tensor("p1", [3, GPC, m1, K], F32, kind="ExternalInput")
    p2f = nc.dram_tensor("p2f", [3, GPC, 1, m2k], F32, kind="ExternalInput")
    out = nc.dram_tensor("out", [3, GPC, 2, m1, m2k], F32,
                         kind="ExternalOutput")

    tiles = [(t, g) for t in range(3) for g in range(GPC)]
    ntile = len(tiles)
    step = ntile / N_PAIRED if N_PAIRED else 0
    paired_set = {int(i * step) for i in range(N_PAIRED)}

    rr = [0]

    with tile.TileContext(nc) as tc:
        with tc.tile_pool(name="in1", bufs=2) as pin1, \
             tc.tile_pool(name="in2", bufs=3) as pin2, \
             tc.tile_pool(name="exp0", bufs=2) as pexp0, \
             tc.tile_pool(name="exp1", bufs=2) as pexp1, \
             tc.tile_pool(name="exp1p", bufs=2) as pexp1p:

            def ring():
                rr[0] += 1
                return nc.sync if rr[0] % 2 else nc.scalar

            def emit_ch0(t, g):
                t1 = pin1.tile([m1, K], F32)
                nc.sync.dma_start(t1[:], p1[t, g])
                e0 = pexp0.tile([m1, m2k], F32)
                nc.vector.tensor_copy(
                    e0[:].rearrange("p (j k) -> p j k", k=K),
                    t1[:].unsqueeze(1).broadcast_to([m1, m2, K]))
                ring().dma_start(out[t, g, 0], e0[:])

            def emit_ch1_natural(t, g):
                t2 = pin2.tile([1, m2k], F32)
                ring().dma_start(t2[:], p2f[t, g])
                e1 = pexp1.tile([m1, m2k], F32)
                nc.gpsimd.partition_broadcast(e1[:], t2[:1])
                ring().dma_start(out[t, g, 1], e1[:])

            def emit_ch1_paired(t, g):
                tp = pin1.tile([npair, 2 * K], F32, tag="tp")
                nc.sync.dma_start(
                    tp[:],
                    p2f[t, g, 0, :paired].rearrange("(p f) -> p f", f=2 * K))
                for i0, i1 in ((0, half1), (half1, m1)):
                    ni = i1 - i0
                    ep = pexp1p.tile([npair, half1 * 2 * K], F32, tag="ep")
                    nc.vector.tensor_copy(
                        ep[:, :ni * 2 * K].rearrange(
                            "p (i f) -> p i f", f=2 * K),
                        tp[:].unsqueeze(1).broadcast_to([npair, ni, 2 * K]))
                    ring().dma_start(
                        out[t, g, 1, i0:i1, :paired].rearrange(
                            "i (p f) -> p i f", f=2 * K),
                        ep[:, :ni * 2 * K].rearrange(
                            "p (i f) -> p i f", f=2 * K))
                if paired < m2k:
                    # odd last j-row: HBM->HBM, stride-0 source, no producer
                    ring().dma_start(
                        out[t, g, 1].rearrange(
                            "i (j k) -> i j k", k=K)[:, m2 - 1],
                        p2f[t, g, 0, paired:].unsqueeze(0).broadcast_to(
                            [m1, K]))

            for idx, (t, g) in enumerate(tiles):
                if idx in paired_set:
                    emit_ch1_paired(t, g)
                else:
                    emit_ch1_natural(t, g)
                emit_ch0(t, g)
    nc.compile()
    return nc


def _pad_groups_np(x, ids, m):
    """[N, K] rows -> [G, m, K] zero-padded per graph (rank within graph)."""
    counts = np.bincount(ids, minlength=G)
    starts = np.cumsum(counts) - counts
    pos = np.arange(ids.shape[0]) - starts[ids]
    outp = np.zeros((G, m, x.shape[1]), x.dtype)
    outp[ids, pos] = x
    return outp


def _make_in_maps(inputs):
    ids1 = np.asarray(inputs['ids1']).astype(np.int64)
    ids2 = np.asarray(inputs['ids2']).astype(np.int64)
    m1 = int(inputs['maxcount1'])
    m2 = int(inputs['maxcount2'])
    xs1 = [np.asarray(inputs[n], dtype=np.float32)
           for n in ('x_f_1', 'x_e_1', 'x_v_1')]
    xs2 = [np.asarray(inputs[n], dtype=np.float32)
           for n in ('x_f_2', 'x_e_2', 'x_v_2')]
    pad1 = np.stack([_pad_groups_np(x, ids1, m1) for x in xs1])
    pad2f = np.stack([_pad_groups_np(x, ids2, m2) for x in xs2]).reshape(
        3, G, 1, m2 * K)
    in_maps = [
        {"p1": np.ascontiguousarray(pad1[:, c * GPC:(c + 1) * GPC]),
         "p2f": np.ascontiguousarray(pad2f[:, c * GPC:(c + 1) * GPC])}
        for c in range(N_CORES)
    ]
    return in_maps, m1, m2


def kernel(**inputs):
    from concourse.bass_utils import run_bass_kernel_spmd

    in_maps, m1, m2 = _make_in_maps(inputs)
    key = (m1, m2)
    if key not in _cache:
        _cache[key] = _build(m1, m2)
    nc = _cache[key]

    res = run_bass_kernel_spmd(nc, in_maps, core_ids=list(range(N_CORES)))

    full = np.empty((3, G, 2, m1, m2, K), np.float32)
    for c in range(N_CORES):
        full[:, c * GPC:(c + 1) * GPC] = res.results[c]["out"].reshape(
            3, GPC, 2, m1, m2, K)
    return full[0], full[1], full[2]


# revision 5
# speedup vs baseline: 25881.3112x; 22936.8211x over previous
import sys

if '/opt/trn_rl_repo' not in sys.path:
    sys.path.insert(0, '/opt/trn_rl_repo')

import numpy as np

G, N1, N2, K = 32, 2048, 2048, 64
N_CORES = 8
GPC = G // N_CORES  # graphs per core

_cache = {}


def _build(m1, m2, opts=None):
    """Per-core Bass program, tuned for this environment's measured rates
    (SWDGE DMA ~170 GB/s with 4-way splits, HWDGE rings ~27 GB/s each,
    DVE copies ~50-100 GB/s and fastest as chains of medium-size copies,
    HBM->HBM DMA fast and engine-free).

      inputs  p1  [3, GPC, m1, K]    group-1 padded rows
              p2f [3, GPC, 1, m2*K]  group-2 padded rows, flattened
      output  out [3, GPC, 2, m1, m2*K]

    ch0 (rows of p1[g] broadcast along j): log-doubling on DVE (8 of 12
    tiles) and ACT (4 of 12) into SBUF tiles (every output element written
    exactly once, copies stay in the fast size range), then 4 gpsimd
    (SWDGE) DMAs per tile to HBM.
    ch1 (p2f[g] repeated on every row i): one stride-0-source HBM->HBM
    DMA per tile straight from the input region - zero engine work.
    """
    from concourse import bacc
    import concourse.tile as tile
    import concourse.mybir as mybir

    opts = opts or {}
    n_act = opts.get("n_act", 4)        # ch0 tiles expanded on ACT
    h2h_splits = opts.get("h2h_splits", 1)
    e0_bufs = opts.get("e0_bufs", 4)
    F32 = mybir.dt.float32
    m2k = m2 * K

    nc = bacc.Bacc("TRN2", target_bir_lowering=False, debug=False)
    p1 = nc.dram_tensor("p1", [3, GPC, m1, K], F32, kind="ExternalInput")
    p2f = nc.dram_tensor("p2f", [3, GPC, 1, m2k], F32, kind="ExternalInput")
    out = nc.dram_tensor("out", [3, GPC, 2, m1, m2k], F32,
                         kind="ExternalOutput")

    with tile.TileContext(nc) as tc:
        with tc.tile_pool(name="in1", bufs=3) as pin1, \
             tc.tile_pool(name="exp0", bufs=e0_bufs) as pexp0:
            idx = 0
            for t in range(3):
                for g in range(GPC):
                    # ---- ch1: stride-0 HBM->HBM, no producer ----
                    q1 = m2k // h2h_splits
                    for c in range(h2h_splits):
                        hi = (c + 1) * q1 if c < h2h_splits - 1 else m2k
                        nc.gpsimd.dma_start(
                            out[t, g, 1, :, c * q1:hi],
                            p2f[t, g, :, c * q1:hi].broadcast_to(
                                [m1, hi - c * q1]))

                    # ---- ch0: engine doubling + 4-split SWDGE DMA ----
                    on_act = n_act > 0 and (idx * n_act) % 12 < n_act
                    idx += 1
                    eng_copy = nc.scalar.copy if on_act else nc.vector.tensor_copy
                    t1 = pin1.tile([m1, K], F32)
                    nc.sync.dma_start(t1[:], p1[t, g])
                    e0 = pexp0.tile([m1, m2k], F32)
                    eng_copy(e0[:, :K], t1[:])
                    reps = 1
                    while reps < m2:
                        n = min(reps, m2 - reps)
                        eng_copy(
                            e0[:, reps * K:(reps + n) * K], e0[:, :n * K])
                        reps += n
                    q = m2k // 4
                    for c in range(4):
                        hi = (c + 1) * q if c < 3 else m2k
                        nc.gpsimd.dma_start(
                            out[t, g, 0, :, c * q:hi], e0[:, c * q:hi])
    nc.compile()
    return nc


def _pad_groups_np(x, ids, m):
    """[N, K] rows -> [G, m, K] zero-padded per graph (rank within graph)."""
    counts = np.bincount(ids, minlength=G)
    starts = np.cumsum(counts) - counts
    pos = np.arange(ids.shape[0]) - starts[ids]
    outp = np.zeros((G, m, x.shape[1]), x.dtype)
    outp[ids, pos] = x
    return outp


def _make_in_maps(inputs):
    ids1 = np.asarray(inputs['ids1']).astype(np.int64)
    ids2 = np.asarray(inputs['ids2']).astype(np.int64)
    m1 = int(inputs['maxcount1'])
    m2 = int(inputs['maxcount2'])
    xs1 = [np.asarray(inputs[n], dtype=np.float32)
           for n in ('x_f_1', 'x_e_1', 'x_v_1')]
    xs2 = [np.asarray(inputs[n], dtype=np.float32)
           for n in ('x_f_2', 'x_e_2', 'x_v_2')]
    pad1 = np.stack([_pad_groups_np(x, ids1, m1) for x in xs1])
    pad2f = np.stack([_pad_groups_np(x, ids2, m2) for x in xs2]).reshape(
        3, G, 1, m2 * K)
    in_maps = [
        {"p1": np.ascontiguousarray(pad1[:, c * GPC:(c + 1) * GPC]),
         "p2f": np.ascontiguousarray(pad2f[:, c * GPC:(c + 1) * GPC])}
        for c in range(N_CORES)
    ]
    return in_maps, m1, m2


def kernel(**inputs):
    from concourse.bass_utils import run_bass_kernel_spmd

    in_maps, m1, m2 = _make_in_maps(inputs)
    key = (m1, m2)
    if key not in _cache:
        _cache[key] = _build(m1, m2)
    nc = _cache[key]

    res = run_bass_kernel_spmd(nc, in_maps, core_ids=list(range(N_CORES)))

    full = np.empty((3, G, 2, m1, m2, K), np.float32)
    for c in range(N_CORES):
        full[:, c * GPC:(c + 1) * GPC] = res.results[c]["out"].reshape(
            3, GPC, 2, m1, m2, K)
    return full[0], full[1], full[2]
